# revision 12
# baseline (speedup 1.0000x reference)
"""Batched CRF Viterbi decode (B=512, T=1024, K=64) on 8 Trainium2 cores.

Strategy (per core, data-parallel over batch: 64 batches/core):
  Forward scan (sequential over T):
    partition layout p in [0,128): p = 64*ih + b holds batch b, i-range
    [32*ih, 32*ih+32). Per step:
      s[p, i, j] = delta_dup[p, j] + T_rep[p, i, j]      (one TT add, 2048/partition)
      mv[p, i]   = max_j s[p, i, j]                       (segmented reduce_max)
      delta_dup' = mv + feat (4 small cross-partition-offset TT adds writing
                   the duplicated layout for the next step)
    mv streams to DRAM (fp32) for the backtrace.
  Backtrace (deferred argmax; no backpointers stored in forward):
    per step t (descending), with tag = path[t+1]:
      onehot = (iota == tag); T-row gather via PE (transpose + matmul, exact
      since weights are 0/1); r = mv_{t-1}+feat_{t-1} + T[tag,:];
      argmax-first via reduce_max -> (r==max)*(j-1024) -> reduce_min -> +1024.
  Exactness: every fp32 add matches the reference op-for-op, so argmax inputs
  are bitwise identical; ties broken to the smallest j (= jnp.argmax).
"""

import numpy as np

import concourse.bacc as bacc
import concourse.mybir as mybir
from concourse import tile

B, T, K = 512, 1024, 64
NCORES = 8
BL = B // NCORES        # 64 batches per core
NI = 32                 # i-values per partition
P = 128
NEG_INF_INIT = -10000.0
CH = 64                 # time chunk
BT_VARIANT = "maxidx"   # full|maxidx|nope (timing ablations)
GS = 16                 # forward i-count computed on GPSIMD (0 = DVE only)

fp32 = mybir.dt.float32
AX = mybir.AxisListType.X
OP = mybir.AluOpType


def _chunks(lo, hi, step=CH):
    """[(t0, n)] covering [lo, hi)"""
    out = []
    t0 = lo
    while t0 < hi:
        n = min(step, hi - t0)
        out.append((t0, n))
        t0 += n
    return out


def build_consts(transitions: np.ndarray) -> dict:
    trep = np.zeros((P, NI * K), np.float32)
    for ih in (0, 1):
        trep[64 * ih : 64 * ih + 64] = transitions[NI * ih : NI * ih + NI, :].reshape(1, -1)
    t32 = np.ascontiguousarray(transitions, np.float32)
    ta32 = (t32.view(np.uint32) & np.uint32(0xFFFF0000)).view(np.float32)
    r1 = (t32 - ta32).astype(np.float32)
    tb32 = (r1.view(np.uint32) & np.uint32(0xFFFF0000)).view(np.float32)
    tc32 = (r1 - tb32).astype(np.float32)
    import ml_dtypes
    ta = ta32.astype(ml_dtypes.bfloat16)
    tb = tb32.astype(ml_dtypes.bfloat16)
    tc = tc32.astype(ml_dtypes.bfloat16)
    # exact 3-way split: parts are bf16-representable and resum to T bitwise
    assert np.array_equal(ta.astype(np.float32), ta32)
    assert np.array_equal(tb.astype(np.float32), tb32)
    assert np.array_equal(tc.astype(np.float32), tc32)
    s01 = ta32 + tb32
    assert np.array_equal((s01 + tc32).astype(np.float32), t32)
    return {
        "trep": trep,
        "ta": ta, "tb": tb, "tc": tc,
        "ident": np.eye(K, dtype=np.float32),
        "iota": np.broadcast_to(np.arange(K, dtype=np.float32), (BL, K)).copy(),
        "jmb": np.broadcast_to(np.arange(K, dtype=np.float32) - 1024.0, (BL, K)).copy(),
        "delta0": np.full((P, K), NEG_INF_INIT, np.float32),
        "mf0": np.full((BL, K), NEG_INF_INIT, np.float32),
    }


def emit_body(nc, tc, d, T_steps, phases=("fwd", "bt")):
    TS = T_steps
    NS = TS - 1

    with tc.tile_pool(name="const", bufs=1) as cpool:
        def load_const(name, shape):
            t = cpool.tile(list(shape), fp32, name=f"{name}_sb")
            nc.sync.dma_start(t[:, :], d[name])
            return t

        trep = load_const("trep", (P, NI * K))
        ident_sb = load_const("ident", (K, K))
        tpack = cpool.tile([K, 3 * K], mybir.dt.bfloat16, name="tpack_sb")
        tpv = tpack[:, :].rearrange("k (j g) -> k j g", g=3)
        for gi, nm in enumerate(("ta", "tb", "tc")):
            nc.sync.dma_start(tpv[:, :, gi], d[nm])
        iota_sb = load_const("iota", (BL, K))
        jmb_sb = load_const("jmb", (BL, K))
        delta0_sb = load_const("delta0", (P, K))
        mf0_sb = load_const("mf0", (BL, K))
        trep3 = trep[:, :].rearrange("p (i k) -> p i k", i=NI)

        path_f32 = cpool.tile([BL, TS], fp32, name="path_f32")
        score_sb = cpool.tile([BL, 1], fp32, name="score_sb")

        # ---------------- forward ----------------
        prev = delta0_sb[:, :]
        with (
            tc.tile_pool(name="fwd", bufs=2) as fpool,
            tc.tile_pool(name="mvp", bufs=2) as mvpool,
        ):
            for (t0, n) in _chunks(1, TS):
                feat_t = fpool.tile([P, n * NI], fp32, tag="feat", name=f"feat{t0}")
                fv = feat_t[:, :].rearrange("p (t k) -> p t k", t=n)
                nc.sync.dma_start(fv[0:64], d["feats"][:, t0 : t0 + n, 0:NI])
                nc.sync.dma_start(fv[64:128], d["feats"][:, t0 : t0 + n, NI:K])
                mvc = mvpool.tile([P, n * NI], fp32, tag="mv", name=f"mv{t0}")
                for j in range(n):
                    s = fpool.tile([P, NI * K], fp32, tag="s", name=f"s{t0 + j}")
                    s3 = s[:, :].rearrange("p (i k) -> p i k", i=NI)
                    DI = NI - GS
                    mv = mvc[:, j * NI : (j + 1) * NI]
                    if GS > 0:
                        db1 = prev.unsqueeze(1).broadcast_to((P, DI, K))
                        db2 = prev.unsqueeze(1).broadcast_to((P, GS, K))
                        nc.vector.tensor_tensor(out=s3[:, 0:DI, :], in0=db1, in1=trep3[:, 0:DI, :], op=OP.add)
                        nc.gpsimd.tensor_tensor(out=s3[:, DI:NI, :], in0=db2, in1=trep3[:, DI:NI, :], op=OP.add)
                        nc.vector.reduce_max(out=mv[:, 0:DI], in_=s3[:, 0:DI, :], axis=AX)
                        nc.vector.reduce_max(out=mv[:, DI:NI], in_=s3[:, DI:NI, :], axis=AX)
                    else:
                        db = prev.unsqueeze(1).broadcast_to((P, NI, K))
                        nc.vector.tensor_tensor(out=s3, in0=db, in1=trep3, op=OP.add)
                        nc.vector.reduce_max(out=mv, in_=s3, axis=AX)
                    dnew = fpool.tile([P, K], fp32, tag="delta", name=f"d{t0 + j}")
                    fc = feat_t[:, j * NI : (j + 1) * NI]
                    nc.vector.tensor_tensor(out=dnew[0:64, 0:NI], in0=mv[0:64], in1=fc[0:64], op=OP.add)
                    nc.vector.tensor_tensor(out=dnew[0:64, NI:K], in0=mv[64:128], in1=fc[64:128], op=OP.add)
                    nc.vector.tensor_tensor(out=dnew[64:128, 0:NI], in0=mv[0:64], in1=fc[0:64], op=OP.add)
                    nc.vector.tensor_tensor(out=dnew[64:128, NI:K], in0=mv[64:128], in1=fc[64:128], op=OP.add)
                    prev = dnew[:, :]
                nc.sync.dma_start(
                    d["mvd"][:, t0 - 1 : t0 - 1 + n, :],
                    mvc[:, :].rearrange("p (t k) -> p t k", t=n),
                )

            # ------------- path_score & last_tag -------------
            nc.vector.reduce_max(out=score_sb[:, 0:1], in_=prev[0:64], axis=AX)
            nc.sync.dma_start(d["score"], score_sb[:, :])
            mk0 = fpool.tile([BL, K], fp32, name="mk0")
            mn0 = fpool.tile([BL, 1], fp32, name="mn0")
            nc.vector.scalar_tensor_tensor(
                out=mk0[:, :], in0=prev[0:64], scalar=score_sb[:, 0:1], in1=jmb_sb[:, :],
                op0=OP.is_equal, op1=OP.mult,
            )
            nc.vector.tensor_reduce(out=mn0[:, 0:1], in_=mk0[:, :], axis=AX, op=OP.min)
            nc.vector.tensor_scalar_add(out=path_f32[:, TS - 1 : TS], in0=mn0[:, 0:1], scalar1=1024.0)

        if "bt" not in phases:
            nc.vector.tensor_copy(out=path_f32[:, 0 : TS - 1], in_=path_f32[:, 1:TS])
            path_i32 = cpool.tile([BL, TS], mybir.dt.int32, name="path_i32")
            nc.vector.tensor_copy(out=path_i32[:, :], in_=path_f32[:, :])
            nc.sync.dma_start(d["path"], path_i32[:, :])
            return
        # ---------------- backtrace ----------------
        with (
            tc.tile_pool(name="bt", bufs=2) as bpool,
            tc.tile_pool(name="btio", bufs=2) as iopool,
            tc.tile_pool(name="ps", bufs=2, space="PSUM") as ppool,
        ):
            def bt_step(u, mf_row, tag_src=None):
                tagcol = tag_src if tag_src is not None else path_f32[:, u + 1 : u + 2]
                r = bpool.tile([BL, K], fp32, tag="r", name=f"r{u}")
                if BT_VARIANT in ("full", "maxidx", "nope"):
                    onehot = bpool.tile([BL, K], mybir.dt.bfloat16, tag="oh", name=f"oh{u}")
                    nc.vector.tensor_scalar(
                        out=onehot[:, :], in0=iota_sb[:, :], scalar1=tagcol, scalar2=None,
                        op0=OP.is_equal,
                    )
                    ohT = bpool.tile([BL, K], mybir.dt.bfloat16, tag="ohT", name=f"ohT{u}")
                    for kb in (0, 1):
                        for bb in (0, 1):
                            nc.vector.transpose(
                                out=ohT[32 * kb : 32 * kb + 32, 32 * bb : 32 * bb + 32],
                                in_=onehot[32 * bb : 32 * bb + 32, 32 * kb : 32 * kb + 32],
                            )
                if BT_VARIANT in ("full", "maxidx"):
                    ptr = ppool.tile([BL, K], fp32, tag="ptr", name=f"ptr{u}")
                    tpv = tpack[:, :].rearrange("k (j g) -> k j g", g=3)
                    nc.tensor.matmul(ptr[:, :], ohT[:, :], tpv[:, :, 0], start=True, stop=False)
                    nc.tensor.matmul(ptr[:, :], ohT[:, :], tpv[:, :, 1], start=False, stop=False)
                    nc.tensor.matmul(ptr[:, :], ohT[:, :], tpv[:, :, 2], start=False, stop=True)
                    nc.vector.tensor_tensor(out=r[:, :], in0=ptr[:, :], in1=mf_row, op=OP.add)
                else:
                    nc.vector.tensor_tensor(out=r[:, :], in0=mf_row, in1=mf_row, op=OP.add)
                if BT_VARIANT == "maxidx":
                    m8 = bpool.tile([BL, 8], fp32, tag="m8", name=f"m8{u}")
                    i8 = bpool.tile([BL, 8], mybir.dt.uint32, tag="i8", name=f"i8{u}")
                    nc.vector.max(out=m8[:, :], in_=r[:, :])
                    nc.vector.max_index(out=i8[:, :], in_max=m8[:, :], in_values=r[:, :])
                    # uint32 -> fp32; doubles as the next step's tag source
                    nc.vector.tensor_copy(out=path_f32[:, u : u + 1], in_=i8[:, 0:1])
                    return None
                mc = bpool.tile([BL, 1], fp32, tag="mc", name=f"mc{u}")
                nc.vector.reduce_max(out=mc[:, 0:1], in_=r[:, :], axis=AX)
                mk = bpool.tile([BL, K], fp32, tag="mk", name=f"mk{u}")
                nc.vector.scalar_tensor_tensor(
                    out=mk[:, :], in0=r[:, :], scalar=mc[:, 0:1], in1=jmb_sb[:, :],
                    op0=OP.is_equal, op1=OP.mult,
                )
                mn = bpool.tile([BL, 1], fp32, tag="mn", name=f"mn{u}")
                nc.vector.tensor_reduce(out=mn[:, 0:1], in_=mk[:, :], axis=AX, op=OP.min)
                nc.vector.tensor_scalar_add(out=path_f32[:, u : u + 1], in0=mn[:, 0:1], scalar1=1024.0)
                return None

            for (u0, n) in reversed(_chunks(1, TS - 1)):
                mv2 = iopool.tile([BL, n * K], fp32, tag="mv2", name=f"mv2_{u0}")
                v = mv2[:, :].rearrange("b (t k) -> b t k", t=n)
                nc.sync.dma_start(v[:, :, 0:NI], d["mvd"][0:64, u0 - 1 : u0 - 1 + n, :])
                nc.sync.dma_start(v[:, :, NI:K], d["mvd"][64:128, u0 - 1 : u0 - 1 + n, :])
                f2 = iopool.tile([BL, n * K], fp32, tag="f2", name=f"f2_{u0}")
                nc.sync.dma_start(
                    f2[:, :].rearrange("b (t k) -> b t k", t=n),
                    d["feats"][:, u0 : u0 + n, :],
                )
                mf = iopool.tile([BL, n * K], fp32, tag="mf", name=f"mf_{u0}")
                nc.vector.tensor_tensor(out=mf[:, :], in0=mv2[:, :], in1=f2[:, :], op=OP.add)
                mfv = mf[:, :].rearrange("b (t k) -> b t k", t=n)
                for j in reversed(range(n)):
                    bt_step(u0 + j, mfv[:, j, :])
            bt_step(0, mf0_sb[:, :])

            path_i32 = cpool.tile([BL, TS], mybir.dt.int32, name="path_i32")
            nc.vector.tensor_copy(out=path_i32[:, :], in_=path_f32[:, :])
            nc.sync.dma_start(d["path"], path_i32[:, :])


def build_program(T_steps=T, phases=("fwd", "bt")):
    nc = bacc.Bacc("TRN2", target_bir_lowering=False, debug=False)
    d = {}
    d["feats"] = nc.dram_tensor("feats", (BL, T_steps, K), fp32, kind="ExternalInput").ap()
    for name, shape in [
        ("trep", (P, NI * K)), ("ident", (K, K)),
        ("iota", (BL, K)), ("jmb", (BL, K)), ("delta0", (P, K)), ("mf0", (BL, K)),
    ]:
        d[name] = nc.dram_tensor(name, shape, fp32, kind="ExternalInput").ap()
    for name in ("ta", "tb", "tc"):
        d[name] = nc.dram_tensor(name, (K, K), mybir.dt.bfloat16, kind="ExternalInput").ap()
    d["score"] = nc.dram_tensor("score", (BL, 1), fp32, kind="ExternalOutput").ap()
    d["path"] = nc.dram_tensor("path", (BL, T_steps), mybir.dt.int32, kind="ExternalOutput").ap()
    d["mvd"] = nc.dram_tensor("mvd", (P, T_steps - 1, NI), fp32, kind="Internal").ap()
    with tile.TileContext(nc) as tc:
        emit_body(nc, tc, d, T_steps, phases=phases)
    nc.compile()
    return nc


def kernel(**inputs):
    from concourse.bass_utils import run_bass_kernel_spmd

    feats = np.ascontiguousarray(np.asarray(inputs["feats"], dtype=np.float32))
    transitions = np.ascontiguousarray(np.asarray(inputs["transitions"], dtype=np.float32))
    consts = build_consts(transitions)
    nc = build_program(T)
    in_maps = [
        {"feats": np.ascontiguousarray(feats[c * BL : (c + 1) * BL]), **consts}
        for c in range(NCORES)
    ]
    res = run_bass_kernel_spmd(nc, in_maps, core_ids=list(range(NCORES)))
    score = np.concatenate([r["score"][:, 0] for r in res.results]).astype(np.float32)
    path = np.concatenate([r["path"] for r in res.results]).astype(np.int32)
    return score, path


# revision 21
# speedup vs baseline: 19.9060x; 19.9060x over previous
"""Batched CRF Viterbi decode (B=512, T=1024, K=64) on 8 Trainium2 cores.

Strategy (per core, data-parallel over batch: 64 batches/core):
  Forward scan (sequential over T, DVE-throughput-bound):
    partition layout p in [0,128): p = 64*ih + b holds batch b, i-range
    [32*ih, 32*ih+32). Per step (i split DVE/GPSIMD at DI = 32-GS):
      s[p, i, j] = delta_dup[p, j] + T_rep[p, i, j]   (TT add: i<DI on DVE,
                                                       i>=DI concurrently on GPSIMD)
      mv[p, i]   = max_j s[p, i, j]                   (two segmented reduce_max on DVE)
      delta_dup' = mv + feat (4 small cross-partition-offset TT adds writing
                   the duplicated layout for the next step)
    mv streams to DRAM (fp32) for the backtrace.
  Backtrace (deferred argmax; no backpointers stored in forward):
    per step t (descending), with tag = path[t+1]:
      onehot = (iota == tag) in bf16; transposed via 4 cross-offset 32x32 DVE
      block transposes; T[tag,:] gathered exactly by 3 accumulating bf16
      matmuls against the truncation-split T = ta+tb+tc (each partial PSUM sum
      exactly representable); r = (mv_{t-1}+feat_{t-1}) + T[tag,:];
      argmax-first via Max8 + MaxIndex (HW-verified first-occurrence ties).
  Exactness: every fp32 add matches the reference op-for-op, so argmax inputs
  are bitwise identical; ties break to the smallest j (= jnp.argmax). Verified
  bit-exact vs the jax reference on hardware (score 512/512, path 524288/524288).
"""

import numpy as np

import concourse.bacc as bacc
import concourse.mybir as mybir
from concourse import tile

B, T, K = 512, 1024, 64
NCORES = 8
BL = B // NCORES        # 64 batches per core
NI = 32                 # i-values per partition
P = 128
NEG_INF_INIT = -10000.0
CH = 64                 # time chunk
BT_VARIANT = "maxidx"   # full|maxidx|nope (timing ablations)
GS = 10                 # forward i-count computed on GPSIMD (0 = DVE only)

fp32 = mybir.dt.float32
AX = mybir.AxisListType.X
OP = mybir.AluOpType


def _chunks(lo, hi, step=CH):
    """[(t0, n)] covering [lo, hi)"""
    out = []
    t0 = lo
    while t0 < hi:
        n = min(step, hi - t0)
        out.append((t0, n))
        t0 += n
    return out


def build_consts(transitions: np.ndarray) -> dict:
    trep = np.zeros((P, NI * K), np.float32)
    for ih in (0, 1):
        trep[64 * ih : 64 * ih + 64] = transitions[NI * ih : NI * ih + NI, :].reshape(1, -1)
    t32 = np.ascontiguousarray(transitions, np.float32)
    ta32 = (t32.view(np.uint32) & np.uint32(0xFFFF0000)).view(np.float32)
    r1 = (t32 - ta32).astype(np.float32)
    tb32 = (r1.view(np.uint32) & np.uint32(0xFFFF0000)).view(np.float32)
    tc32 = (r1 - tb32).astype(np.float32)
    import ml_dtypes
    ta = ta32.astype(ml_dtypes.bfloat16)
    tb = tb32.astype(ml_dtypes.bfloat16)
    tc = tc32.astype(ml_dtypes.bfloat16)
    # exact 3-way split: parts are bf16-representable and resum to T bitwise
    assert np.array_equal(ta.astype(np.float32), ta32)
    assert np.array_equal(tb.astype(np.float32), tb32)
    assert np.array_equal(tc.astype(np.float32), tc32)
    s01 = ta32 + tb32
    assert np.array_equal((s01 + tc32).astype(np.float32), t32)
    return {
        "trep": trep,
        "ta": ta, "tb": tb, "tc": tc,
        "ident": np.eye(K, dtype=np.float32),
        "iota": np.broadcast_to(np.arange(K, dtype=np.float32), (BL, K)).copy(),
        "jmb": np.broadcast_to(np.arange(K, dtype=np.float32) - 1024.0, (BL, K)).copy(),
        "delta0": np.full((P, K), NEG_INF_INIT, np.float32),
        "mf0": np.full((BL, K), NEG_INF_INIT, np.float32),
    }


def emit_body(nc, tc, d, T_steps, phases=("fwd", "bt")):
    TS = T_steps
    NS = TS - 1

    with tc.tile_pool(name="const", bufs=1) as cpool:
        def load_const(name, shape):
            t = cpool.tile(list(shape), fp32, name=f"{name}_sb")
            nc.sync.dma_start(t[:, :], d[name])
            return t

        trep = load_const("trep", (P, NI * K))
        ident_sb = load_const("ident", (K, K))
        tpack = cpool.tile([K, 3 * K], mybir.dt.bfloat16, name="tpack_sb")
        tpv = tpack[:, :].rearrange("k (j g) -> k j g", g=3)
        for gi, nm in enumerate(("ta", "tb", "tc")):
            nc.sync.dma_start(tpv[:, :, gi], d[nm])
        iota_sb = load_const("iota", (BL, K))
        jmb_sb = load_const("jmb", (BL, K))
        delta0_sb = load_const("delta0", (P, K))
        mf0_sb = load_const("mf0", (BL, K))
        trep3 = trep[:, :].rearrange("p (i k) -> p i k", i=NI)

        path_f32 = cpool.tile([BL, TS], fp32, name="path_f32")
        ibuf = cpool.tile([BL, 8 * TS], mybir.dt.uint32, name="ibuf")
        score_sb = cpool.tile([BL, 1], fp32, name="score_sb")

        # ---------------- forward ----------------
        prev = delta0_sb[:, :]
        with (
            tc.tile_pool(name="fwd", bufs=2) as fpool,
            tc.tile_pool(name="mvp", bufs=2) as mvpool,
        ):
            for (t0, n) in _chunks(1, TS):
                feat_t = fpool.tile([P, n * NI], fp32, tag="feat", name=f"feat{t0}")
                fv = feat_t[:, :].rearrange("p (t k) -> p t k", t=n)
                nc.sync.dma_start(fv[0:64], d["feats"][:, t0 : t0 + n, 0:NI])
                nc.sync.dma_start(fv[64:128], d["feats"][:, t0 : t0 + n, NI:K])
                mvc = mvpool.tile([P, n * NI], fp32, tag="mv", name=f"mv{t0}")
                for j in range(n):
                    s = fpool.tile([P, NI * K], fp32, tag="s", name=f"s{t0 + j}")
                    s3 = s[:, :].rearrange("p (i k) -> p i k", i=NI)
                    DI = NI - GS
                    mv = mvc[:, j * NI : (j + 1) * NI]
                    if GS > 0:
                        db1 = prev.unsqueeze(1).broadcast_to((P, DI, K))
                        db2 = prev.unsqueeze(1).broadcast_to((P, GS, K))
                        nc.vector.tensor_tensor(out=s3[:, 0:DI, :], in0=db1, in1=trep3[:, 0:DI, :], op=OP.add)
                        nc.gpsimd.tensor_tensor(out=s3[:, DI:NI, :], in0=db2, in1=trep3[:, DI:NI, :], op=OP.add)
                        nc.vector.reduce_max(out=mv[:, 0:DI], in_=s3[:, 0:DI, :], axis=AX)
                        nc.vector.reduce_max(out=mv[:, DI:NI], in_=s3[:, DI:NI, :], axis=AX)
                    else:
                        db = prev.unsqueeze(1).broadcast_to((P, NI, K))
                        nc.vector.tensor_tensor(out=s3, in0=db, in1=trep3, op=OP.add)
                        nc.vector.reduce_max(out=mv, in_=s3, axis=AX)
                    dnew = fpool.tile([P, K], fp32, tag="delta", name=f"d{t0 + j}")
                    fc = feat_t[:, j * NI : (j + 1) * NI]
                    nc.vector.tensor_tensor(out=dnew[0:64, 0:NI], in0=mv[0:64], in1=fc[0:64], op=OP.add)
                    nc.vector.tensor_tensor(out=dnew[0:64, NI:K], in0=mv[64:128], in1=fc[64:128], op=OP.add)
                    nc.vector.tensor_tensor(out=dnew[64:128, 0:NI], in0=mv[0:64], in1=fc[0:64], op=OP.add)
                    nc.vector.tensor_tensor(out=dnew[64:128, NI:K], in0=mv[64:128], in1=fc[64:128], op=OP.add)
                    prev = dnew[:, :]
                nc.sync.dma_start(
                    d["mvd"][:, t0 - 1 : t0 - 1 + n, :],
                    mvc[:, :].rearrange("p (t k) -> p t k", t=n),
                )

            # ------------- path_score & last_tag -------------
            nc.vector.reduce_max(out=score_sb[:, 0:1], in_=prev[0:64], axis=AX)
            nc.sync.dma_start(d["score"], score_sb[:, :])
            mk0 = fpool.tile([BL, K], fp32, name="mk0")
            mn0 = fpool.tile([BL, 1], fp32, name="mn0")
            nc.vector.scalar_tensor_tensor(
                out=mk0[:, :], in0=prev[0:64], scalar=score_sb[:, 0:1], in1=jmb_sb[:, :],
                op0=OP.is_equal, op1=OP.mult,
            )
            nc.vector.tensor_reduce(out=mn0[:, 0:1], in_=mk0[:, :], axis=AX, op=OP.min)
            nc.vector.tensor_scalar_add(out=path_f32[:, TS - 1 : TS], in0=mn0[:, 0:1], scalar1=1024.0)

        if "bt" not in phases:
            nc.vector.tensor_copy(out=path_f32[:, 0 : TS - 1], in_=path_f32[:, 1:TS])
            path_i32 = cpool.tile([BL, TS], mybir.dt.int32, name="path_i32")
            if BT_VARIANT == "maxidx":
                iv = ibuf[:, :].rearrange("b (t e) -> b t e", e=8)
                nc.vector.tensor_copy(out=path_i32[:, 0 : TS - 1], in_=iv[:, 0 : TS - 1, 0])
                nc.vector.tensor_copy(out=path_i32[:, TS - 1 : TS], in_=path_f32[:, TS - 1 : TS])
            else:
                nc.vector.tensor_copy(out=path_i32[:, :], in_=path_f32[:, :])
            nc.sync.dma_start(d["path"], path_i32[:, :])
            return
        # ---------------- backtrace ----------------
        with (
            tc.tile_pool(name="bt", bufs=2) as bpool,
            tc.tile_pool(name="btio", bufs=2) as iopool,
            tc.tile_pool(name="ps", bufs=2, space="PSUM") as ppool,
        ):
            def bt_half(u, mf_row, h):
                """One backtrace step for batch half h (partitions 32h:32h+32).
                Two independent chains overlap: one chain's DVE ops hide the
                other's PE round-trip."""
                lo, hi = 32 * h, 32 * h + 32
                tagb = ibuf[lo:hi, 8 * (u + 1) : 8 * (u + 1) + 1].broadcast_to((32, K))
                onehot = bpool.tile([32, K], mybir.dt.bfloat16, tag=f"oh{h}", name=f"oh{h}_{u}")
                nc.vector.tensor_tensor(
                    out=onehot[:, :], in0=iota_sb[lo:hi, :], in1=tagb, op=OP.is_equal,
                )
                ohT = bpool.tile([K, 32], mybir.dt.bfloat16, tag=f"ohT{h}", name=f"ohT{h}_{u}")
                for kb in (0, 1):
                    nc.vector.transpose(
                        out=ohT[32 * kb : 32 * kb + 32, 0:32],
                        in_=onehot[0:32, 32 * kb : 32 * kb + 32],
                    )
                ptr = ppool.tile([32, K], fp32, tag=f"ptr{h}", name=f"ptr{h}_{u}")
                tpv = tpack[:, :].rearrange("k (j g) -> k j g", g=3)
                nc.tensor.matmul(ptr[:, :], ohT[:, :], tpv[:, :, 0], start=True, stop=False)
                nc.tensor.matmul(ptr[:, :], ohT[:, :], tpv[:, :, 1], start=False, stop=False)
                nc.tensor.matmul(ptr[:, :], ohT[:, :], tpv[:, :, 2], start=False, stop=True)
                r = bpool.tile([32, K], fp32, tag=f"r{h}", name=f"r{h}_{u}")
                nc.vector.tensor_tensor(out=r[:, :], in0=ptr[:, :], in1=mf_row, op=OP.add)
                m8 = bpool.tile([32, 8], fp32, tag=f"m8{h}", name=f"m8{h}_{u}")
                nc.vector.max(out=m8[:, :], in_=r[:, :])
                nc.vector.max_index(
                    out=ibuf[lo:hi, 8 * u : 8 * u + 8], in_max=m8[:, :], in_values=r[:, :],
                )

            def bt_step(u, mf_row, tag_src=None):
                if BT_VARIANT == "maxidx":
                    mf3 = mf_row
                    bt_half(u, mf3[0:32, :], 0)
                    bt_half(u, mf3[32:64, :], 1)
                    return None
                r = bpool.tile([BL, K], fp32, tag="r", name=f"r{u}")
                if BT_VARIANT in ("full", "nope"):
                    onehot = bpool.tile([BL, K], mybir.dt.bfloat16, tag="oh", name=f"oh{u}")
                    tagcol = tag_src if tag_src is not None else path_f32[:, u + 1 : u + 2]
                    nc.vector.tensor_scalar(
                        out=onehot[:, :], in0=iota_sb[:, :], scalar1=tagcol, scalar2=None,
                        op0=OP.is_equal,
                    )
                    ohT = bpool.tile([BL, K], mybir.dt.bfloat16, tag="ohT", name=f"ohT{u}")
                    for kb in (0, 1):
                        for bb in (0, 1):
                            nc.vector.transpose(
                                out=ohT[32 * kb : 32 * kb + 32, 32 * bb : 32 * bb + 32],
                                in_=onehot[32 * bb : 32 * bb + 32, 32 * kb : 32 * kb + 32],
                            )
                if BT_VARIANT == "full":
                    ptr = ppool.tile([BL, K], fp32, tag="ptr", name=f"ptr{u}")
                    tpv = tpack[:, :].rearrange("k (j g) -> k j g", g=3)
                    nc.tensor.matmul(ptr[:, :], ohT[:, :], tpv[:, :, 0], start=True, stop=False)
                    nc.tensor.matmul(ptr[:, :], ohT[:, :], tpv[:, :, 1], start=False, stop=False)
                    nc.tensor.matmul(ptr[:, :], ohT[:, :], tpv[:, :, 2], start=False, stop=True)
                    nc.vector.tensor_tensor(out=r[:, :], in0=ptr[:, :], in1=mf_row, op=OP.add)
                else:
                    nc.vector.tensor_tensor(out=r[:, :], in0=mf_row, in1=mf_row, op=OP.add)
                mc = bpool.tile([BL, 1], fp32, tag="mc", name=f"mc{u}")
                nc.vector.reduce_max(out=mc[:, 0:1], in_=r[:, :], axis=AX)
                mk = bpool.tile([BL, K], fp32, tag="mk", name=f"mk{u}")
                nc.vector.scalar_tensor_tensor(
                    out=mk[:, :], in0=r[:, :], scalar=mc[:, 0:1], in1=jmb_sb[:, :],
                    op0=OP.is_equal, op1=OP.mult,
                )
                mn = bpool.tile([BL, 1], fp32, tag="mn", name=f"mn{u}")
                nc.vector.tensor_reduce(out=mn[:, 0:1], in_=mk[:, :], axis=AX, op=OP.min)
                nc.vector.tensor_scalar_add(out=path_f32[:, u : u + 1], in0=mn[:, 0:1], scalar1=1024.0)
                return None

            if BT_VARIANT == "maxidx":
                nc.vector.tensor_copy(
                    out=ibuf[:, 8 * (TS - 1) : 8 * (TS - 1) + 1], in_=path_f32[:, TS - 1 : TS],
                )
            for (u0, n) in reversed(_chunks(1, TS - 1)):
                mv2 = iopool.tile([BL, n * K], fp32, tag="mv2", name=f"mv2_{u0}")
                v = mv2[:, :].rearrange("b (t k) -> b t k", t=n)
                nc.sync.dma_start(v[:, :, 0:NI], d["mvd"][0:64, u0 - 1 : u0 - 1 + n, :])
                nc.sync.dma_start(v[:, :, NI:K], d["mvd"][64:128, u0 - 1 : u0 - 1 + n, :])
                f2 = iopool.tile([BL, n * K], fp32, tag="f2", name=f"f2_{u0}")
                nc.sync.dma_start(
                    f2[:, :].rearrange("b (t k) -> b t k", t=n),
                    d["feats"][:, u0 : u0 + n, :],
                )
                mf = iopool.tile([BL, n * K], fp32, tag="mf", name=f"mf_{u0}")
                nc.gpsimd.tensor_tensor(out=mf[:, :], in0=mv2[:, :], in1=f2[:, :], op=OP.add)
                mfv = mf[:, :].rearrange("b (t k) -> b t k", t=n)
                for j in reversed(range(n)):
                    bt_step(u0 + j, mfv[:, j, :])
            bt_step(0, mf0_sb[:, :])

            path_i32 = cpool.tile([BL, TS], mybir.dt.int32, name="path_i32")
            if BT_VARIANT == "maxidx":
                iv = ibuf[:, :].rearrange("b (t e) -> b t e", e=8)
                nc.vector.tensor_copy(out=path_i32[:, 0 : TS - 1], in_=iv[:, 0 : TS - 1, 0])
                nc.vector.tensor_copy(out=path_i32[:, TS - 1 : TS], in_=path_f32[:, TS - 1 : TS])
            else:
                nc.vector.tensor_copy(out=path_i32[:, :], in_=path_f32[:, :])
            nc.sync.dma_start(d["path"], path_i32[:, :])


def build_program(T_steps=T, phases=("fwd", "bt")):
    nc = bacc.Bacc("TRN2", target_bir_lowering=False, debug=False)
    d = {}
    d["feats"] = nc.dram_tensor("feats", (BL, T_steps, K), fp32, kind="ExternalInput").ap()
    for name, shape in [
        ("trep", (P, NI * K)), ("ident", (K, K)),
        ("iota", (BL, K)), ("jmb", (BL, K)), ("delta0", (P, K)), ("mf0", (BL, K)),
    ]:
        d[name] = nc.dram_tensor(name, shape, fp32, kind="ExternalInput").ap()
    for name in ("ta", "tb", "tc"):
        d[name] = nc.dram_tensor(name, (K, K), mybir.dt.bfloat16, kind="ExternalInput").ap()
    d["score"] = nc.dram_tensor("score", (BL, 1), fp32, kind="ExternalOutput").ap()
    d["path"] = nc.dram_tensor("path", (BL, T_steps), mybir.dt.int32, kind="ExternalOutput").ap()
    d["mvd"] = nc.dram_tensor("mvd", (P, T_steps - 1, NI), fp32, kind="Internal").ap()
    with tile.TileContext(nc) as tc:
        emit_body(nc, tc, d, T_steps, phases=phases)
    nc.compile()
    return nc


_PROG_CACHE = {}


def kernel(**inputs):
    from concourse.bass_utils import run_bass_kernel_spmd

    feats = np.ascontiguousarray(np.asarray(inputs["feats"], dtype=np.float32))
    transitions = np.ascontiguousarray(np.asarray(inputs["transitions"], dtype=np.float32))
    consts = build_consts(transitions)
    if "nc" not in _PROG_CACHE:
        _PROG_CACHE["nc"] = build_program(T)
    nc = _PROG_CACHE["nc"]
    in_maps = [
        {"feats": np.ascontiguousarray(feats[c * BL : (c + 1) * BL]), **consts}
        for c in range(NCORES)
    ]
    res = run_bass_kernel_spmd(nc, in_maps, core_ids=list(range(NCORES)))
    score = np.concatenate([r["score"][:, 0] for r in res.results]).astype(np.float32)
    path = np.concatenate([r["path"] for r in res.results]).astype(np.int32)
    return score, path


# revision 24
# speedup vs baseline: 20.1987x; 1.0147x over previous
"""Batched CRF Viterbi decode (B=512, T=1024, K=64) on 8 Trainium2 cores.

Strategy (per core, data-parallel over batch: 64 batches/core):
  Forward scan (sequential over T, DVE-throughput-bound):
    partition layout p in [0,128): p = 64*ih + b holds batch b, i-range
    [32*ih, 32*ih+32). Per step (i split DVE/GPSIMD at DI = 32-GS):
      s[p, i, j] = delta_dup[p, j] + T_rep[p, i, j]   (TT add: i<DI on DVE,
                                                       i>=DI concurrently on GPSIMD)
      mv[p, i]   = max_j s[p, i, j]                   (two segmented reduce_max on DVE)
      delta_dup' = mv + feat (4 small cross-partition-offset TT adds writing
                   the duplicated layout for the next step)
    mv streams to DRAM (fp32) for the backtrace.
  Backtrace (deferred argmax; no backpointers stored in forward):
    per step t (descending), with tag = path[t+1]:
      onehot = (iota == tag) in bf16; transposed via 4 cross-offset 32x32 DVE
      block transposes; T[tag,:] gathered exactly by 3 accumulating bf16
      matmuls against the truncation-split T = ta+tb+tc (each partial PSUM sum
      exactly representable); r = (mv_{t-1}+feat_{t-1}) + T[tag,:];
      argmax-first via Max8 + MaxIndex (HW-verified first-occurrence ties).
  Exactness: every fp32 add matches the reference op-for-op, so argmax inputs
  are bitwise identical; ties break to the smallest j (= jnp.argmax). Verified
  bit-exact vs the jax reference on hardware (score 512/512, path 524288/524288).
"""

import numpy as np

import concourse.bacc as bacc
import concourse.mybir as mybir
from concourse import tile

B, T, K = 512, 1024, 64
NCORES = 8
BL = B // NCORES        # 64 batches per core
NI = 32                 # i-values per partition
P = 128
NEG_INF_INIT = -10000.0
CH = 64                 # time chunk
BT_VARIANT = "maxidx"   # full|maxidx|nope (timing ablations)
GS = 10                 # forward i-count computed on GPSIMD (0 = DVE only)

fp32 = mybir.dt.float32
AX = mybir.AxisListType.X
OP = mybir.AluOpType


def _chunks(lo, hi, step=CH):
    """[(t0, n)] covering [lo, hi)"""
    out = []
    t0 = lo
    while t0 < hi:
        n = min(step, hi - t0)
        out.append((t0, n))
        t0 += n
    return out


def build_consts(transitions: np.ndarray) -> dict:
    trep = np.zeros((P, NI * K), np.float32)
    for ih in (0, 1):
        trep[64 * ih : 64 * ih + 64] = transitions[NI * ih : NI * ih + NI, :].reshape(1, -1)
    t32 = np.ascontiguousarray(transitions, np.float32)
    ta32 = (t32.view(np.uint32) & np.uint32(0xFFFF0000)).view(np.float32)
    r1 = (t32 - ta32).astype(np.float32)
    tb32 = (r1.view(np.uint32) & np.uint32(0xFFFF0000)).view(np.float32)
    tc32 = (r1 - tb32).astype(np.float32)
    import ml_dtypes
    ta = ta32.astype(ml_dtypes.bfloat16)
    tb = tb32.astype(ml_dtypes.bfloat16)
    tc = tc32.astype(ml_dtypes.bfloat16)
    # exact 3-way split: parts are bf16-representable and resum to T bitwise
    assert np.array_equal(ta.astype(np.float32), ta32)
    assert np.array_equal(tb.astype(np.float32), tb32)
    assert np.array_equal(tc.astype(np.float32), tc32)
    s01 = ta32 + tb32
    assert np.array_equal((s01 + tc32).astype(np.float32), t32)
    return {
        "trep": trep,
        "ta": ta, "tb": tb, "tc": tc,
        "ident": np.eye(K, dtype=np.float32),
        "iota": np.broadcast_to(np.arange(K, dtype=np.float32), (BL, K)).copy(),
        "jmb": np.broadcast_to(np.arange(K, dtype=np.float32) - 1024.0, (BL, K)).copy(),
        "delta0": np.full((P, K), NEG_INF_INIT, np.float32),
        "mf0": np.full((BL, K), NEG_INF_INIT, np.float32),
    }


def emit_body(nc, tc, d, T_steps, phases=("fwd", "bt")):
    TS = T_steps
    NS = TS - 1

    with tc.tile_pool(name="const", bufs=1) as cpool:
        def load_const(name, shape):
            t = cpool.tile(list(shape), fp32, name=f"{name}_sb")
            nc.sync.dma_start(t[:, :], d[name])
            return t

        trep = load_const("trep", (P, NI * K))
        ident_sb = load_const("ident", (K, K))
        tpack = cpool.tile([K, 3 * K], mybir.dt.bfloat16, name="tpack_sb")
        tpv = tpack[:, :].rearrange("k (j g) -> k j g", g=3)
        for gi, nm in enumerate(("ta", "tb", "tc")):
            nc.sync.dma_start(tpv[:, :, gi], d[nm])
        iota_sb = load_const("iota", (BL, K))
        jmb_sb = load_const("jmb", (BL, K))
        delta0_sb = load_const("delta0", (P, K))
        mf0_sb = load_const("mf0", (BL, K))
        trep3 = trep[:, :].rearrange("p (i k) -> p i k", i=NI)

        path_f32 = cpool.tile([BL, TS], fp32, name="path_f32")
        ibuf = cpool.tile([BL, 8 * TS], mybir.dt.uint32, name="ibuf")
        score_sb = cpool.tile([BL, 1], fp32, name="score_sb")

        # ---------------- forward ----------------
        prev = delta0_sb[:, :]
        with (
            tc.tile_pool(name="fwd", bufs=2) as fpool,
            tc.tile_pool(name="mvp", bufs=2) as mvpool,
        ):
            for (t0, n) in _chunks(1, TS):
                feat_t = fpool.tile([P, n * NI], fp32, tag="feat", name=f"feat{t0}")
                fv = feat_t[:, :].rearrange("p (t k) -> p t k", t=n)
                nc.sync.dma_start(fv[0:64], d["feats"][:, t0 : t0 + n, 0:NI])
                nc.sync.dma_start(fv[64:128], d["feats"][:, t0 : t0 + n, NI:K])
                mvc = mvpool.tile([P, n * NI], fp32, tag="mv", name=f"mv{t0}")
                for j in range(n):
                    s = fpool.tile([P, NI * K], fp32, tag="s", name=f"s{t0 + j}")
                    s3 = s[:, :].rearrange("p (i k) -> p i k", i=NI)
                    DI = NI - GS
                    mv = mvc[:, j * NI : (j + 1) * NI]
                    if GS > 0:
                        db1 = prev.unsqueeze(1).broadcast_to((P, DI, K))
                        db2 = prev.unsqueeze(1).broadcast_to((P, GS, K))
                        nc.vector.tensor_tensor(out=s3[:, 0:DI, :], in0=db1, in1=trep3[:, 0:DI, :], op=OP.add)
                        nc.gpsimd.tensor_tensor(out=s3[:, DI:NI, :], in0=db2, in1=trep3[:, DI:NI, :], op=OP.add)
                        nc.vector.reduce_max(out=mv[:, 0:DI], in_=s3[:, 0:DI, :], axis=AX)
                        nc.vector.reduce_max(out=mv[:, DI:NI], in_=s3[:, DI:NI, :], axis=AX)
                    else:
                        db = prev.unsqueeze(1).broadcast_to((P, NI, K))
                        nc.vector.tensor_tensor(out=s3, in0=db, in1=trep3, op=OP.add)
                        nc.vector.reduce_max(out=mv, in_=s3, axis=AX)
                    dnew = fpool.tile([P, K], fp32, tag="delta", name=f"d{t0 + j}")
                    fc = feat_t[:, j * NI : (j + 1) * NI]
                    nc.vector.tensor_tensor(out=dnew[0:64, 0:NI], in0=mv[0:64], in1=fc[0:64], op=OP.add)
                    nc.vector.tensor_tensor(out=dnew[0:64, NI:K], in0=mv[64:128], in1=fc[64:128], op=OP.add)
                    nc.gpsimd.tensor_tensor(out=dnew[64:128, 0:NI], in0=mv[0:64], in1=fc[0:64], op=OP.add)
                    nc.gpsimd.tensor_tensor(out=dnew[64:128, NI:K], in0=mv[64:128], in1=fc[64:128], op=OP.add)
                    prev = dnew[:, :]
                nc.sync.dma_start(
                    d["mvd"][:, t0 - 1 : t0 - 1 + n, :],
                    mvc[:, :].rearrange("p (t k) -> p t k", t=n),
                )

            # ------------- path_score & last_tag -------------
            nc.vector.reduce_max(out=score_sb[:, 0:1], in_=prev[0:64], axis=AX)
            nc.sync.dma_start(d["score"], score_sb[:, :])
            mk0 = fpool.tile([BL, K], fp32, name="mk0")
            mn0 = fpool.tile([BL, 1], fp32, name="mn0")
            nc.vector.scalar_tensor_tensor(
                out=mk0[:, :], in0=prev[0:64], scalar=score_sb[:, 0:1], in1=jmb_sb[:, :],
                op0=OP.is_equal, op1=OP.mult,
            )
            nc.vector.tensor_reduce(out=mn0[:, 0:1], in_=mk0[:, :], axis=AX, op=OP.min)
            nc.vector.tensor_scalar_add(out=path_f32[:, TS - 1 : TS], in0=mn0[:, 0:1], scalar1=1024.0)

        if "bt" not in phases:
            nc.vector.tensor_copy(out=path_f32[:, 0 : TS - 1], in_=path_f32[:, 1:TS])
            path_i32 = cpool.tile([BL, TS], mybir.dt.int32, name="path_i32")
            if BT_VARIANT == "maxidx":
                iv = ibuf[:, :].rearrange("b (t e) -> b t e", e=8)
                nc.vector.tensor_copy(out=path_i32[:, 0 : TS - 1], in_=iv[:, 0 : TS - 1, 0])
                nc.vector.tensor_copy(out=path_i32[:, TS - 1 : TS], in_=path_f32[:, TS - 1 : TS])
            else:
                nc.vector.tensor_copy(out=path_i32[:, :], in_=path_f32[:, :])
            nc.sync.dma_start(d["path"], path_i32[:, :])
            return
        # ---------------- backtrace ----------------
        with (
            tc.tile_pool(name="bt", bufs=2) as bpool,
            tc.tile_pool(name="btio", bufs=2) as iopool,
            tc.tile_pool(name="ps", bufs=2, space="PSUM") as ppool,
        ):
            def bt_half(u, mf_row, h):
                """One backtrace step for batch half h (partitions 32h:32h+32).
                Two independent chains overlap: one chain's DVE ops hide the
                other's PE round-trip."""
                lo, hi = 32 * h, 32 * h + 32
                tagb = ibuf[lo:hi, 8 * (u + 1) : 8 * (u + 1) + 1].broadcast_to((32, K))
                onehot = bpool.tile([32, K], mybir.dt.bfloat16, tag=f"oh{h}", name=f"oh{h}_{u}")
                nc.vector.tensor_tensor(
                    out=onehot[:, :], in0=iota_sb[lo:hi, :], in1=tagb, op=OP.is_equal,
                )
                ohT = bpool.tile([K, 32], mybir.dt.bfloat16, tag=f"ohT{h}", name=f"ohT{h}_{u}")
                for kb in (0, 1):
                    nc.vector.transpose(
                        out=ohT[32 * kb : 32 * kb + 32, 0:32],
                        in_=onehot[0:32, 32 * kb : 32 * kb + 32],
                    )
                ptr = ppool.tile([32, K], fp32, tag=f"ptr{h}", name=f"ptr{h}_{u}")
                tpv = tpack[:, :].rearrange("k (j g) -> k j g", g=3)
                nc.tensor.matmul(ptr[:, :], ohT[:, :], tpv[:, :, 0], start=True, stop=False)
                nc.tensor.matmul(ptr[:, :], ohT[:, :], tpv[:, :, 1], start=False, stop=False)
                nc.tensor.matmul(ptr[:, :], ohT[:, :], tpv[:, :, 2], start=False, stop=True)
                r = bpool.tile([32, K], fp32, tag=f"r{h}", name=f"r{h}_{u}")
                nc.vector.tensor_tensor(out=r[:, :], in0=ptr[:, :], in1=mf_row, op=OP.add)
                m8 = bpool.tile([32, 8], fp32, tag=f"m8{h}", name=f"m8{h}_{u}")
                nc.vector.max(out=m8[:, :], in_=r[:, :])
                nc.vector.max_index(
                    out=ibuf[lo:hi, 8 * u : 8 * u + 8], in_max=m8[:, :], in_values=r[:, :],
                )

            def bt_step(u, mf_row, tag_src=None):
                if BT_VARIANT == "maxidx":
                    mf3 = mf_row
                    bt_half(u, mf3[0:32, :], 0)
                    bt_half(u, mf3[32:64, :], 1)
                    return None
                r = bpool.tile([BL, K], fp32, tag="r", name=f"r{u}")
                if BT_VARIANT in ("full", "nope"):
                    onehot = bpool.tile([BL, K], mybir.dt.bfloat16, tag="oh", name=f"oh{u}")
                    tagcol = tag_src if tag_src is not None else path_f32[:, u + 1 : u + 2]
                    nc.vector.tensor_scalar(
                        out=onehot[:, :], in0=iota_sb[:, :], scalar1=tagcol, scalar2=None,
                        op0=OP.is_equal,
                    )
                    ohT = bpool.tile([BL, K], mybir.dt.bfloat16, tag="ohT", name=f"ohT{u}")
                    for kb in (0, 1):
                        for bb in (0, 1):
                            nc.vector.transpose(
                                out=ohT[32 * kb : 32 * kb + 32, 32 * bb : 32 * bb + 32],
                                in_=onehot[32 * bb : 32 * bb + 32, 32 * kb : 32 * kb + 32],
                            )
                if BT_VARIANT == "full":
                    ptr = ppool.tile([BL, K], fp32, tag="ptr", name=f"ptr{u}")
                    tpv = tpack[:, :].rearrange("k (j g) -> k j g", g=3)
                    nc.tensor.matmul(ptr[:, :], ohT[:, :], tpv[:, :, 0], start=True, stop=False)
                    nc.tensor.matmul(ptr[:, :], ohT[:, :], tpv[:, :, 1], start=False, stop=False)
                    nc.tensor.matmul(ptr[:, :], ohT[:, :], tpv[:, :, 2], start=False, stop=True)
                    nc.vector.tensor_tensor(out=r[:, :], in0=ptr[:, :], in1=mf_row, op=OP.add)
                else:
                    nc.vector.tensor_tensor(out=r[:, :], in0=mf_row, in1=mf_row, op=OP.add)
                mc = bpool.tile([BL, 1], fp32, tag="mc", name=f"mc{u}")
                nc.vector.reduce_max(out=mc[:, 0:1], in_=r[:, :], axis=AX)
                mk = bpool.tile([BL, K], fp32, tag="mk", name=f"mk{u}")
                nc.vector.scalar_tensor_tensor(
                    out=mk[:, :], in0=r[:, :], scalar=mc[:, 0:1], in1=jmb_sb[:, :],
                    op0=OP.is_equal, op1=OP.mult,
                )
                mn = bpool.tile([BL, 1], fp32, tag="mn", name=f"mn{u}")
                nc.vector.tensor_reduce(out=mn[:, 0:1], in_=mk[:, :], axis=AX, op=OP.min)
                nc.vector.tensor_scalar_add(out=path_f32[:, u : u + 1], in0=mn[:, 0:1], scalar1=1024.0)
                return None

            if BT_VARIANT == "maxidx":
                nc.vector.tensor_copy(
                    out=ibuf[:, 8 * (TS - 1) : 8 * (TS - 1) + 1], in_=path_f32[:, TS - 1 : TS],
                )
            for (u0, n) in reversed(_chunks(1, TS - 1)):
                mv2 = iopool.tile([BL, n * K], fp32, tag="mv2", name=f"mv2_{u0}")
                v = mv2[:, :].rearrange("b (t k) -> b t k", t=n)
                nc.sync.dma_start(v[:, :, 0:NI], d["mvd"][0:64, u0 - 1 : u0 - 1 + n, :])
                nc.sync.dma_start(v[:, :, NI:K], d["mvd"][64:128, u0 - 1 : u0 - 1 + n, :])
                f2 = iopool.tile([BL, n * K], fp32, tag="f2", name=f"f2_{u0}")
                nc.sync.dma_start(
                    f2[:, :].rearrange("b (t k) -> b t k", t=n),
                    d["feats"][:, u0 : u0 + n, :],
                )
                mf = iopool.tile([BL, n * K], fp32, tag="mf", name=f"mf_{u0}")
                nc.gpsimd.tensor_tensor(out=mf[:, :], in0=mv2[:, :], in1=f2[:, :], op=OP.add)
                mfv = mf[:, :].rearrange("b (t k) -> b t k", t=n)
                for j in reversed(range(n)):
                    bt_step(u0 + j, mfv[:, j, :])
            bt_step(0, mf0_sb[:, :])

            path_i32 = cpool.tile([BL, TS], mybir.dt.int32, name="path_i32")
            if BT_VARIANT == "maxidx":
                iv = ibuf[:, :].rearrange("b (t e) -> b t e", e=8)
                nc.vector.tensor_copy(out=path_i32[:, 0 : TS - 1], in_=iv[:, 0 : TS - 1, 0])
                nc.vector.tensor_copy(out=path_i32[:, TS - 1 : TS], in_=path_f32[:, TS - 1 : TS])
            else:
                nc.vector.tensor_copy(out=path_i32[:, :], in_=path_f32[:, :])
            nc.sync.dma_start(d["path"], path_i32[:, :])


def build_program(T_steps=T, phases=("fwd", "bt")):
    nc = bacc.Bacc("TRN2", target_bir_lowering=False, debug=False)
    d = {}
    d["feats"] = nc.dram_tensor("feats", (BL, T_steps, K), fp32, kind="ExternalInput").ap()
    for name, shape in [
        ("trep", (P, NI * K)), ("ident", (K, K)),
        ("iota", (BL, K)), ("jmb", (BL, K)), ("delta0", (P, K)), ("mf0", (BL, K)),
    ]:
        d[name] = nc.dram_tensor(name, shape, fp32, kind="ExternalInput").ap()
    for name in ("ta", "tb", "tc"):
        d[name] = nc.dram_tensor(name, (K, K), mybir.dt.bfloat16, kind="ExternalInput").ap()
    d["score"] = nc.dram_tensor("score", (BL, 1), fp32, kind="ExternalOutput").ap()
    d["path"] = nc.dram_tensor("path", (BL, T_steps), mybir.dt.int32, kind="ExternalOutput").ap()
    d["mvd"] = nc.dram_tensor("mvd", (P, T_steps - 1, NI), fp32, kind="Internal").ap()
    with tile.TileContext(nc) as tc:
        emit_body(nc, tc, d, T_steps, phases=phases)
    nc.compile()
    return nc


_PROG_CACHE = {}


def kernel(**inputs):
    from concourse.bass_utils import run_bass_kernel_spmd

    feats = np.ascontiguousarray(np.asarray(inputs["feats"], dtype=np.float32))
    transitions = np.ascontiguousarray(np.asarray(inputs["transitions"], dtype=np.float32))
    consts = build_consts(transitions)
    if "nc" not in _PROG_CACHE:
        _PROG_CACHE["nc"] = build_program(T)
    nc = _PROG_CACHE["nc"]
    in_maps = [
        {"feats": np.ascontiguousarray(feats[c * BL : (c + 1) * BL]), **consts}
        for c in range(NCORES)
    ]
    res = run_bass_kernel_spmd(nc, in_maps, core_ids=list(range(NCORES)))
    score = np.concatenate([r["score"][:, 0] for r in res.results]).astype(np.float32)
    path = np.concatenate([r["path"] for r in res.results]).astype(np.int32)
    return score, path
